# revision 10
# baseline (speedup 1.0000x reference)
"""DGANet dual-GAT layer on 8 Trainium2 NeuronCores (Bass/Tile), v2.

Math (per branch b in {n, d}):
    Wh = h @ W_b                                  [4096, 256]
    e  = leaky_relu(s1_i + s2_j, 0.2)             s1 = h@(W@a1), s2 = h@(W@a2)
    att = softmax(where(adj>0, e, -9e15), axis=-1)
    f_b = elu(att @ Wh)
Output: f_n + f_d.

Sharding: 1D row-parallel over the 4096 attention rows (512 rows/core).
Each core computes the full Wh and holds its score block transposed,
P^T[j, i] (j on partitions), so att @ Wh contracts over j on the tensor
engine.  The adjacency mask is a host-prepared additive bias (0 or -16384,
fp16): exp underflows masked entries to exactly 0.

v2 vs v1:
  - fp16 end-to-end on the data path (hT/W/masks shipped fp16 from host):
    no fp32->fp32r conversion copies, half the DMA bytes, and the DVE runs
    its 2-byte 2x/4x perf modes on the logit elementwise ops.
  - row-sum of exp moved off the tensor engine: pt tiles are accumulated on
    DVE (delayed by one pair so the DVE queue never blocks the exp chain)
    and a single ones-matmul per branch reduces across partitions.
  - single flat produce/consume stream across both branches (no inter-branch
    PE bubble), epilogue emitted mid-stream.
  - fewer, larger DMAs (packed wt/a/W tiles), masks interleaved with hT
    chunks so the attention pipeline starts ~4us in.
  - output written as [F, R] straight from SBUF (no PE transposes); host
    transposes the per-core block.
"""

from contextlib import ExitStack

import numpy as np

import concourse.bass as bass
import concourse.bacc as bacc
import concourse.mybir as mybir
import concourse.tile as tile
from concourse import bass_utils

N, FIN, F = 4096, 512, 256
NCORES = 8
R = N // NCORES            # 512 attention rows per core
P = 128                    # partitions
NJT = N // P               # 32 j-tiles
NKT = FIN // P             # 4 fin contraction tiles
NPR = NJT // 2             # 16 j-tile pairs
CH = 4                     # hT column chunks
CHW = N // CH              # 1024 j-cols per chunk
MASKB = -16384.0           # additive mask: exp underflows to 0
ALPHA = 0.2
DELAY = 2                  # produce->consume pipeline depth (in pairs)

F32 = mybir.dt.float32
F16 = mybir.dt.float16
AF = mybir.ActivationFunctionType
ALU = mybir.AluOpType
BR = ("n", "d")


def build_program(reps=None):
    """reps=None: single-shot program (grading path).  reps=K: body wrapped
    in a K-iteration hardware loop, for wall-clock HW timing by slope."""
    nc = bacc.Bacc("TRN2", target_bir_lowering=False, debug=False,
                   num_devices=NCORES)

    hT = nc.dram_tensor("ht", [FIN, N], F16, kind="ExternalInput").ap()
    hTo = nc.dram_tensor("hto", [FIN, R], F16, kind="ExternalInput").ap()
    W = {b: nc.dram_tensor(f"w_{b}", [FIN, F], F16, kind="ExternalInput").ap()
         for b in BR}
    WT = {b: nc.dram_tensor(f"wt_{b}", [F, FIN], F16, kind="ExternalInput").ap()
          for b in BR}
    A = {b: nc.dram_tensor(f"a_{b}", [F, 2], F16, kind="ExternalInput").ap()
         for b in BR}
    MT = {b: nc.dram_tensor(f"mt_{b}", [N, R], F16, kind="ExternalInput").ap()
          for b in BR}
    OUT = nc.dram_tensor("out", [F, R], F32, kind="ExternalOutput").ap()

    with tile.TileContext(nc) as tc:
        if reps is None:
            with ExitStack() as ctx:
                _body(ctx, nc, tc, hT, hTo, W, WT, A, MT, OUT)
        else:
            with tc.For_i(0, reps, 1,
                          hint_engines=(mybir.EngineType.PE,)):
                with ExitStack() as ctx:
                    _body(ctx, nc, tc, hT, hTo, W, WT, A, MT, OUT)
    nc.compile()
    return nc


def _body(ctx, nc, tc, hT, hTo, W, WT, A, MT, OUT):
    consts = ctx.enter_context(tc.tile_pool(name="consts", bufs=1))
    # PSUM banks (8): pp_work 4 + 4 acc banks; rsps borrows a pp_work slot
    pp_work = ctx.enter_context(tc.tile_pool(name="pp_work", bufs=4,
                                             space="PSUM"))
    pp_acc = ctx.enter_context(tc.tile_pool(name="pp_acc", bufs=1,
                                            space="PSUM"))
    maskp = ctx.enter_context(tc.tile_pool(name="maskp", bufs=1))
    whp = ctx.enter_context(tc.tile_pool(name="whp", bufs=6))
    workp = ctx.enter_context(tc.tile_pool(name="workp", bufs=4))
    pexp = ctx.enter_context(tc.tile_pool(name="pexp", bufs=4))
    epip = ctx.enter_context(tc.tile_pool(name="epip", bufs=2))

    ones16 = consts.tile([P, R], F16, tag="ones16")
    nc.vector.memset(ones16, 1.0)

    # PE warm-up on resident constants: ~3.5us so the HAM clock gate reaches
    # 2.4 GHz while the first hT/weight DMAs are still streaming.
    wps = pp_work.tile([P, R], F32, tag="pswork", name="wps")
    for _ in range(8):
        nc.tensor.matmul(wps, lhsT=ones16[:, 0:P], rhs=ones16,
                         start=True, stop=True)

    # ---- DMA issue order (one serialized HWDGE stream) ---------------------
    # needed first: ht chunk 0, branch-n weights, hto; masks interleave after.
    ht_sb = {}

    def dma_ht(ch):
        for k in range(NKT):
            t = consts.tile([P, CHW], F16, tag=f"ht{k}_{ch}")
            nc.sync.dma_start(
                out=t, in_=hT[k * P:(k + 1) * P, ch * CHW:(ch + 1) * CHW])
            ht_sb[k, ch] = t

    dma_ht(0)

    wsb = {}
    wt_sb = {}
    a_sb = {}
    for b in BR:
        t = consts.tile([P, NKT, F], F16, tag=f"w_{b}")
        nc.sync.dma_start(
            out=t, in_=W[b].rearrange("(kt p) f -> p kt f", p=P))
        wsb[b] = t
        wt = consts.tile([P, 2, FIN], F16, tag=f"wt_{b}")
        nc.sync.dma_start(
            out=wt, in_=WT[b].rearrange("(two p) fin -> p two fin", p=P))
        wt_sb[b] = wt
        at = consts.tile([P, 2, 2], F16, tag=f"a_{b}")
        nc.sync.dma_start(
            out=at, in_=A[b].rearrange("(two p) c -> p two c", p=P))
        a_sb[b] = at
        if b == "n":
            hto_t = consts.tile([P, NKT, R], F16, tag="hto")
            nc.sync.dma_start(
                out=hto_t, in_=hTo.rearrange("(kt p) r -> p kt r", p=P))

    mask_t = {}

    def dma_mask(b, tp):
        m = maskp.tile([P, 2 * R], F16, tag=f"m_{b}{tp}", name=f"m{tp}")
        nc.sync.dma_start(
            out=m.rearrange("p (two r) -> p two r", two=2),
            in_=MT[b][2 * tp * P:(2 * tp + 2) * P, :].rearrange(
                "(two p) r -> p two r", two=2))
        mask_t[b, tp] = m

    for tp in range(2):
        dma_mask("n", tp)
    dma_ht(1)
    for tp in range(2, 6):
        dma_mask("n", tp)
    dma_ht(2)
    for tp in range(6, 12):
        dma_mask("n", tp)
    dma_ht(3)
    for tp in range(12, NPR):
        dma_mask("n", tp)
    for tp in range(NPR):
        dma_mask("d", tp)

    # ---- small weight prep: wa = W@a on PE, s1b ----------------------------
    wa_r = {}
    wa16 = {}
    for b in BR:
        ps = pp_work.tile([P, 2 * NKT], F32, tag="pswork", name="wa")
        for m in range(NKT):
            for fk in range(2):
                nc.tensor.matmul(
                    ps[:, 2 * m:2 * m + 2],
                    lhsT=wt_sb[b][:, fk, m * P:(m + 1) * P],
                    rhs=a_sb[b][:, fk, :],
                    start=(fk == 0), stop=(fk == 1))
        wa = consts.tile([P, 2 * NKT], F32, tag=f"wa_{b}")
        nc.vector.tensor_copy(out=wa, in_=ps)
        w16 = consts.tile([P, 2 * NKT], F16, tag=f"wa16_{b}")
        nc.vector.tensor_copy(out=w16, in_=wa)
        wa16[b] = w16
        # wa1 chunk m replicated across 128 cols: stationary operand whose
        # matmul output is s1 already broadcast over partitions.
        reps = []
        for m in range(NKT):
            rt = consts.tile([P, P], F16, tag=f"war_{b}{m}", name=f"war{m}")
            nc.vector.tensor_copy(
                out=rt, in_=wa[:, 2 * m:2 * m + 1].broadcast_to((P, P)))
            reps.append(rt)
        wa_r[b] = reps

    s1b = {}
    for b in BR:
        ps1 = pp_work.tile([P, R], F32, tag="pswork", name="s1")
        for k in range(NKT):
            nc.tensor.matmul(
                ps1, lhsT=wa_r[b][k], rhs=hto_t[:, k, :],
                start=(k == 0), stop=(k == NKT - 1))
        t = consts.tile([P, R], F16, tag=f"s1b_{b}")
        nc.scalar.copy(out=t, in_=ps1)
        s1b[b] = t

    # ---- main loop: flat produce/consume stream over both branches ---------
    acc = {(b, fh): pp_acc.tile([P, R], F32, tag=f"acc_{b}{fh}",
                                name=f"acc_{b}{fh}")
           for b in BR for fh in range(2)}
    racc = {b: workp.tile([P, R], F16, tag=f"racc_{b}", name="racc", bufs=1)
            for b in BR}
    tb = {"n": [None, None], "d": [None, None]}

    def emit_racc(b, tp, ptp):
        # delayed one pair behind produce so these adds queue after the next
        # pair's ump/lrp on DVE.  Pool pre-adds the two halves (TensorTensor
        # is the one elementwise op GPSIMD supports, SBUF-only).
        tmp = workp.tile([P, R], F16, tag="rtmp", name="rtmp", bufs=2)
        nc.gpsimd.tensor_tensor(out=tmp, in0=ptp[:, 0:R], in1=ptp[:, R:2 * R],
                                op=ALU.add)
        if tp == 0:
            nc.vector.tensor_copy(out=racc[b], in_=tmp)
        else:
            nc.vector.tensor_tensor(out=racc[b], in0=racc[b], in1=tmp,
                                    op=ALU.add)

    def produce(b, tp):
        jt0 = 2 * tp
        ch, off = divmod(jt0 * P, CHW)
        # Wh pair in ONE psum bank [P, 2, F]; s2 pair in a second bank.
        ps = pp_work.tile([P, 2, F], F32, tag="pswork", name="ps")
        s2ps = pp_work.tile([P, 2], F32, tag="pswork", name="s2ps")
        for half in range(2):
            for k in range(NKT):
                lhsT = ht_sb[k, ch][:, off + half * P:off + (half + 1) * P]
                nc.tensor.matmul(
                    ps[:, half, :], lhsT=lhsT, rhs=wsb[b][:, k, :],
                    start=(k == 0), stop=(k == NKT - 1))
                nc.tensor.matmul(
                    s2ps[:, half:half + 1], lhsT=lhsT,
                    rhs=wa16[b][:, 2 * k + 1:2 * k + 2],
                    start=(k == 0), stop=(k == NKT - 1))
        wh = whp.tile([P, 2, F], F16, tag="wh", name="wh")
        nc.scalar.copy(out=wh, in_=ps)
        s2 = whp.tile([P, 2], F32, tag="s2", name="s2", bufs=6)
        nc.scalar.copy(out=s2, in_=s2ps)

        # masked logits u = s1 + s2 + maskbias; leaky_relu; exp.  The stt ops
        # run on DVE (TTSS is 2x for packed fp16 on real HW); exp+copies on
        # Act; Pool only does legal SBUF TensorTensor work (racc pre-adds).
        m = mask_t[b, tp]
        ump = workp.tile([P, 2, R], F16, tag="ump", name="ump", bufs=3)
        for half in range(2):
            nc.vector.scalar_tensor_tensor(
                out=ump[:, half, :], in0=s1b[b], scalar=s2[:, half:half + 1],
                in1=m[:, half * R:(half + 1) * R], op0=ALU.add, op1=ALU.add)
        lrp = workp.tile([P, 2 * R], F16, tag="lrp", name="lrp", bufs=3)
        umf = ump.rearrange("p two r -> p (two r)")
        nc.vector.scalar_tensor_tensor(
            out=lrp, in0=umf, scalar=ALPHA, in1=umf,
            op0=ALU.mult, op1=ALU.max)
        ptp = pexp.tile([P, 2 * R], F16, tag="ptp", name="ptp")
        nc.scalar.activation(out=ptp, in_=lrp, func=AF.Exp)
        whs = [wh[:, 0, :], wh[:, 1, :]]
        return whs, ptp

    def consume(b, tp, whs, ptp):
        first, last = (tp == 0), (tp == NPR - 1)
        for half in range(2):
            pt = ptp[:, half * R:(half + 1) * R]
            st = first and half == 0
            sp = last and half == 1
            nc.tensor.matmul(acc[b, 0], lhsT=whs[half][:, 0:P], rhs=pt,
                             start=st, stop=sp)
            nc.tensor.matmul(acc[b, 1], lhsT=whs[half][:, P:F], rhs=pt,
                             start=st, stop=sp)

    def epilogue(b):
        rsps = pp_work.tile([P, R], F32, tag="pswork", name="rsps")
        nc.tensor.matmul(rsps, lhsT=ones16[:, 0:P], rhs=racc[b],
                         start=True, stop=True)
        rb = epip.tile([P, R], F32, tag="rb", name="rb", bufs=1)
        nc.vector.reciprocal(out=rb, in_=rsps)
        for fh in range(2):
            o = epip.tile([P, R], F32, tag="o", name="o")
            nc.vector.scalar_tensor_tensor(
                out=o, in0=acc[b, fh], scalar=1.0, in1=rb,
                op0=ALU.mult, op1=ALU.mult)
            rl = epip.tile([P, R], F32, tag="rl", name="rl", bufs=1)
            nc.scalar.activation(out=rl, in_=o, func=AF.Relu)
            em = epip.tile([P, R], F32, tag="em", name="em", bufs=1)
            nc.scalar.activation(out=em, in_=o, func=AF.Exp)
            t = epip.tile([P, R], F32, tag=f"t_{b}{fh}", name="t", bufs=1)
            # t = min(exp(o), 1) + relu(o)  ==  elu(o) + 1
            nc.vector.scalar_tensor_tensor(
                out=t, in0=em, scalar=1.0, in1=rl, op0=ALU.min, op1=ALU.add)
            tb[b][fh] = t

    items = [(b, tp) for b in BR for tp in range(NPR)]
    inflight = []
    prev_pt = {}
    for b, tp in items:
        inflight.append((b, tp, *produce(b, tp)))
        if tp > 0:
            emit_racc(b, tp - 1, prev_pt[b])
        prev_pt[b] = inflight[-1][3]
        if len(inflight) > DELAY:
            bc, tpc, whs, ptp = inflight.pop(0)
            consume(bc, tpc, whs, ptp)
            if tpc == NPR - 1:
                emit_racc(bc, NPR - 1, prev_pt[bc])
                epilogue(bc)
    for bc, tpc, whs, ptp in inflight:
        consume(bc, tpc, whs, ptp)
        if tpc == NPR - 1:
            emit_racc(bc, NPR - 1, prev_pt[bc])
            epilogue(bc)

    for fh in range(2):
        c = epip.tile([P, R], F32, tag="comb", name="comb")
        # c = (t_n - 2) + t_d  ==  elu(o_n) + elu(o_d)
        nc.vector.scalar_tensor_tensor(
            out=c, in0=tb["n"][fh], scalar=-2.0, in1=tb["d"][fh],
            op0=ALU.add, op1=ALU.add)
        nc.sync.dma_start(out=OUT[fh * P:(fh + 1) * P, :], in_=c)


_CACHED = None


def _get_program():
    global _CACHED
    if _CACHED is None:
        _CACHED = build_program()
    return _CACHED


def _prep_inputs(h, adj_n, adj_d, W_n, a1_n, a2_n, W_d, a1_d, a2_d):
    h = np.asarray(h, np.float32)
    hT = np.ascontiguousarray(h.T)
    com = {
        "ht": hT.astype(np.float16),
        "w_n": np.asarray(W_n, np.float32).astype(np.float16),
        "w_d": np.asarray(W_d, np.float32).astype(np.float16),
        "wt_n": np.ascontiguousarray(
            np.asarray(W_n, np.float32).T).astype(np.float16),
        "wt_d": np.ascontiguousarray(
            np.asarray(W_d, np.float32).T).astype(np.float16),
        "a_n": np.concatenate(
            [np.asarray(a1_n, np.float32), np.asarray(a2_n, np.float32)],
            axis=1).astype(np.float16),
        "a_d": np.concatenate(
            [np.asarray(a1_d, np.float32), np.asarray(a2_d, np.float32)],
            axis=1).astype(np.float16),
    }
    adj = {"n": np.asarray(adj_n), "d": np.asarray(adj_d)}
    maps = []
    for c in range(NCORES):
        m = dict(com)
        m["hto"] = np.ascontiguousarray(
            hT[:, c * R:(c + 1) * R]).astype(np.float16)
        for b in BR:
            blk = adj[b][c * R:(c + 1) * R, :]          # [R, N]
            mt = np.where(blk.T > 0, np.float16(0.0), np.float16(MASKB))
            m[f"mt_{b}"] = np.ascontiguousarray(mt.astype(np.float16))
        maps.append(m)
    return maps


def run_on_hw(inputs, trace=False):
    nc = _get_program()
    maps = _prep_inputs(
        inputs["h"], inputs["adj_n"], inputs["adj_d"],
        inputs["W_n"], inputs["a1_n"], inputs["a2_n"],
        inputs["W_d"], inputs["a1_d"], inputs["a2_d"])
    last_err = None
    for attempt in range(3):
        try:
            res = bass_utils.run_bass_kernel_spmd(
                nc, maps, core_ids=list(range(NCORES)), trace=trace)
            break
        except Exception as e:          # transient NRT/axon failures recover
            last_err = e
            import time as _time
            _time.sleep(5)
    else:
        raise last_err
    out = np.concatenate(
        [res.results[c]["out"].T for c in range(NCORES)], axis=0)
    return np.ascontiguousarray(out), res


def kernel(**inputs):
    out, _ = run_on_hw(inputs, trace=False)
    return out


# revision 12
# speedup vs baseline: 1.3334x; 1.3334x over previous
"""DGANet dual-GAT layer on 8 Trainium2 NeuronCores (Bass/Tile), v2.

Math (per branch b in {n, d}):
    Wh = h @ W_b                                  [4096, 256]
    e  = leaky_relu(s1_i + s2_j, 0.2)             s1 = h@(W@a1), s2 = h@(W@a2)
    att = softmax(where(adj>0, e, -9e15), axis=-1)
    f_b = elu(att @ Wh)
Output: f_n + f_d.

Sharding: 1D row-parallel over the 4096 attention rows (512 rows/core).
Each core computes the full Wh and holds its score block transposed,
P^T[j, i] (j on partitions), so att @ Wh contracts over j on the tensor
engine.  The adjacency mask is a host-prepared additive bias (0 or -16384,
fp16): exp underflows masked entries to exactly 0.

v2 vs v1:
  - fp16 end-to-end on the data path (hT/W/masks shipped fp16 from host):
    no fp32->fp32r conversion copies, half the DMA bytes, and the DVE runs
    its 2-byte 2x/4x perf modes on the logit elementwise ops.
  - row-sum of exp moved off the tensor engine: pt tiles are accumulated on
    DVE (delayed by one pair so the DVE queue never blocks the exp chain)
    and a single ones-matmul per branch reduces across partitions.
  - single flat produce/consume stream across both branches (no inter-branch
    PE bubble), epilogue emitted mid-stream.
  - fewer, larger DMAs (packed wt/a/W tiles), masks interleaved with hT
    chunks so the attention pipeline starts ~4us in.
  - output written as [F, R] straight from SBUF (no PE transposes); host
    transposes the per-core block.
"""

from contextlib import ExitStack

import numpy as np

import concourse.bass as bass
import concourse.bacc as bacc
import concourse.mybir as mybir
import concourse.tile as tile
from concourse import bass_utils
import concourse.dve_ops as _D
import concourse.dve_spec as _dsp
from concourse.dve_spec import Spec as _Spec, Src0 as _S0, Src1 as _S1, \
    C0 as _C0, C2 as _C2, One as _One, maxx as _maxx, minn as _minn, \
    relu as _relu, lower as _lower
from concourse.dve_uop import DveOpSpec as _DveOpSpec


def _register_dve_op(name, spec):
    """Register a custom DVE op (idempotent) and return it."""
    for _o in _D.OPS:
        if _o.name == name:
            return _o
    _D._SUB_OPCODE_FOR_NAME[name] = _D._CUSTOM_DVE_ROW_BASE + len(_D.OPS)
    assert _D._SUB_OPCODE_FOR_NAME[name] < 0x20
    has_src1 = _S1 in _dsp.spec_leaves(spec)
    shas = {}
    for _ver in ("v3", "v4"):
        _s = _DveOpSpec(name=name, opcode=_D.get_dve_sub_opcode(name),
                        uops=_lower(spec, ver=_ver), rd1_en=has_src1)
        shas[_ver] = _s.sha(_ver)
    op = _D.DveOp(name, spec, subdim=False, uops_sha=shas)
    _D.OPS.append(op)
    return op


_U = _S0 + _S1 + _C0
# out = leaky_relu(in0 + in1 + s0, imm2): the whole masked-logit elementwise
# (s1 + mask + s2 then leaky) in one DVE instruction.
LOGIT_OP = _register_dve_op("LOGIT_FUSED_DG", _Spec(
    body=_maxx(_U, _U * _C2),
    reference=lambda in0, in1, s0, s1, imm2:
        __import__("numpy").maximum(in0 + in1 + s0,
                                    (in0 + in1 + s0) * imm2).astype("float32")))
# out = min(in0, 1) + relu(in1)  ==  elu(in1) + 1 when in0 == exp(in1)
ELU1_OP = _register_dve_op("ELU1_FUSED_DG", _Spec(
    body=_minn(_S0, _One) + _relu(_S1),
    reference=lambda in0, in1, s0, s1, imm2:
        (__import__("numpy").minimum(in0, 1.0)
         + __import__("numpy").maximum(in1, 0.0)).astype("float32")))

N, FIN, F = 4096, 512, 256
NCORES = 8
R = N // NCORES            # 512 attention rows per core
P = 128                    # partitions
NJT = N // P               # 32 j-tiles
NKT = FIN // P             # 4 fin contraction tiles
NPR = NJT // 2             # 16 j-tile pairs
CH = 4                     # hT column chunks
CHW = N // CH              # 1024 j-cols per chunk
MASKB = -16384.0           # additive mask: exp underflows to 0
ALPHA = 0.2
DELAY = 2                  # produce->consume pipeline depth (in pairs)

F32 = mybir.dt.float32
F16 = mybir.dt.float16
AF = mybir.ActivationFunctionType
ALU = mybir.AluOpType
BR = ("n", "d")


def build_program(reps=None):
    """reps=None: single-shot program (grading path).  reps=K: body wrapped
    in a K-iteration hardware loop, for wall-clock HW timing by slope."""
    nc = bacc.Bacc("TRN2", target_bir_lowering=False, debug=False,
                   num_devices=NCORES)

    hT = nc.dram_tensor("ht", [FIN, N], F16, kind="ExternalInput").ap()
    hTo = nc.dram_tensor("hto", [FIN, R], F16, kind="ExternalInput").ap()
    W = {b: nc.dram_tensor(f"w_{b}", [FIN, F], F16, kind="ExternalInput").ap()
         for b in BR}
    WT = {b: nc.dram_tensor(f"wt_{b}", [F, FIN], F16, kind="ExternalInput").ap()
          for b in BR}
    A = {b: nc.dram_tensor(f"a_{b}", [F, 2], F16, kind="ExternalInput").ap()
         for b in BR}
    MT = {b: nc.dram_tensor(f"mt_{b}", [N, R], F16, kind="ExternalInput").ap()
          for b in BR}
    OUT = nc.dram_tensor("out", [F, R], F32, kind="ExternalOutput").ap()

    with tile.TileContext(nc) as tc:
        if reps is None:
            with ExitStack() as ctx:
                _body(ctx, nc, tc, hT, hTo, W, WT, A, MT, OUT)
        else:
            with tc.For_i(0, reps, 1,
                          hint_engines=(mybir.EngineType.PE,)):
                with ExitStack() as ctx:
                    _body(ctx, nc, tc, hT, hTo, W, WT, A, MT, OUT)
    nc.compile()
    return nc


def _body(ctx, nc, tc, hT, hTo, W, WT, A, MT, OUT):
    consts = ctx.enter_context(tc.tile_pool(name="consts", bufs=1))
    # PSUM banks (8): pp_work 4 + 4 acc banks; rsps borrows a pp_work slot
    pp_work = ctx.enter_context(tc.tile_pool(name="pp_work", bufs=4,
                                             space="PSUM"))
    pp_acc = ctx.enter_context(tc.tile_pool(name="pp_acc", bufs=1,
                                            space="PSUM"))
    maskp = ctx.enter_context(tc.tile_pool(name="maskp", bufs=1))
    whp = ctx.enter_context(tc.tile_pool(name="whp", bufs=6))
    workp = ctx.enter_context(tc.tile_pool(name="workp", bufs=4))
    pexp = ctx.enter_context(tc.tile_pool(name="pexp", bufs=4))
    epip = ctx.enter_context(tc.tile_pool(name="epip", bufs=2))

    ones16 = consts.tile([P, R], F16, tag="ones16")
    nc.vector.memset(ones16, 1.0)

    # PE warm-up on resident constants: ~3.5us so the HAM clock gate reaches
    # 2.4 GHz while the first hT/weight DMAs are still streaming.
    wps = pp_work.tile([P, R], F32, tag="pswork", name="wps")
    for _ in range(8):
        nc.tensor.matmul(wps, lhsT=ones16[:, 0:P], rhs=ones16,
                         start=True, stop=True)

    # ---- DMA issue order (one serialized HWDGE stream) ---------------------
    # needed first: ht chunk 0, branch-n weights, hto; masks interleave after.
    ht_sb = {}

    def dma_ht(ch):
        for k in range(NKT):
            t = consts.tile([P, CHW], F16, tag=f"ht{k}_{ch}")
            nc.sync.dma_start(
                out=t, in_=hT[k * P:(k + 1) * P, ch * CHW:(ch + 1) * CHW])
            ht_sb[k, ch] = t

    dma_ht(0)

    wsb = {}
    wt_sb = {}
    a_sb = {}
    for b in BR:
        t = consts.tile([P, NKT, F], F16, tag=f"w_{b}")
        nc.sync.dma_start(
            out=t, in_=W[b].rearrange("(kt p) f -> p kt f", p=P))
        wsb[b] = t
        wt = consts.tile([P, 2, FIN], F16, tag=f"wt_{b}")
        nc.sync.dma_start(
            out=wt, in_=WT[b].rearrange("(two p) fin -> p two fin", p=P))
        wt_sb[b] = wt
        at = consts.tile([P, 2, 2], F16, tag=f"a_{b}")
        nc.sync.dma_start(
            out=at, in_=A[b].rearrange("(two p) c -> p two c", p=P))
        a_sb[b] = at
        if b == "n":
            hto_t = consts.tile([P, NKT, R], F16, tag="hto")
            nc.sync.dma_start(
                out=hto_t, in_=hTo.rearrange("(kt p) r -> p kt r", p=P))

    mask_t = {}

    def dma_mask(b, tp):
        m = maskp.tile([P, 2 * R], F16, tag=f"m_{b}{tp}", name=f"m{tp}")
        nc.sync.dma_start(
            out=m.rearrange("p (two r) -> p two r", two=2),
            in_=MT[b][2 * tp * P:(2 * tp + 2) * P, :].rearrange(
                "(two p) r -> p two r", two=2))
        mask_t[b, tp] = m

    for tp in range(2):
        dma_mask("n", tp)
    dma_ht(1)
    for tp in range(2, 6):
        dma_mask("n", tp)
    dma_ht(2)
    for tp in range(6, 12):
        dma_mask("n", tp)
    dma_ht(3)
    for tp in range(12, NPR):
        dma_mask("n", tp)
    for tp in range(NPR):
        dma_mask("d", tp)

    # ---- small weight prep: wa = W@a on PE, s1b ----------------------------
    wa_r = {}
    wa16 = {}
    for b in BR:
        ps = pp_work.tile([P, 2 * NKT], F32, tag="pswork", name="wa")
        for m in range(NKT):
            for fk in range(2):
                nc.tensor.matmul(
                    ps[:, 2 * m:2 * m + 2],
                    lhsT=wt_sb[b][:, fk, m * P:(m + 1) * P],
                    rhs=a_sb[b][:, fk, :],
                    start=(fk == 0), stop=(fk == 1))
        wa = consts.tile([P, 2 * NKT], F32, tag=f"wa_{b}")
        nc.vector.tensor_copy(out=wa, in_=ps)
        w16 = consts.tile([P, 2 * NKT], F16, tag=f"wa16_{b}")
        nc.vector.tensor_copy(out=w16, in_=wa)
        wa16[b] = w16
        # wa1 chunk m replicated across 128 cols: stationary operand whose
        # matmul output is s1 already broadcast over partitions.
        reps = []
        for m in range(NKT):
            rt = consts.tile([P, P], F16, tag=f"war_{b}{m}", name=f"war{m}")
            nc.vector.tensor_copy(
                out=rt, in_=wa[:, 2 * m:2 * m + 1].broadcast_to((P, P)))
            reps.append(rt)
        wa_r[b] = reps

    s1b = {}
    for b in BR:
        ps1 = pp_work.tile([P, R], F32, tag="pswork", name="s1")
        for k in range(NKT):
            nc.tensor.matmul(
                ps1, lhsT=wa_r[b][k], rhs=hto_t[:, k, :],
                start=(k == 0), stop=(k == NKT - 1))
        t = consts.tile([P, R], F16, tag=f"s1b_{b}")
        nc.scalar.copy(out=t, in_=ps1)
        s1b[b] = t

    # ---- main loop: flat produce/consume stream over both branches ---------
    acc = {(b, fh): pp_acc.tile([P, R], F32, tag=f"acc_{b}{fh}",
                                name=f"acc_{b}{fh}")
           for b in BR for fh in range(2)}
    racc = {b: workp.tile([P, R], F16, tag=f"racc_{b}", name="racc", bufs=1)
            for b in BR}
    tb = {"n": [None, None], "d": [None, None]}

    def emit_racc(b, tp, ptp):
        # delayed one pair behind produce so these adds queue after the next
        # pair's ump/lrp on DVE.  Pool pre-adds the two halves (TensorTensor
        # is the one elementwise op GPSIMD supports, SBUF-only).
        tmp = workp.tile([P, R], F16, tag="rtmp", name="rtmp", bufs=2)
        nc.gpsimd.tensor_tensor(out=tmp, in0=ptp[:, 0:R], in1=ptp[:, R:2 * R],
                                op=ALU.add)
        if tp == 0:
            nc.vector.tensor_copy(out=racc[b], in_=tmp)
        else:
            nc.vector.tensor_tensor(out=racc[b], in0=racc[b], in1=tmp,
                                    op=ALU.add)

    def produce(b, tp):
        jt0 = 2 * tp
        ch, off = divmod(jt0 * P, CHW)
        # Wh pair in ONE psum bank [P, 2, F]; s2 pair in a second bank.
        ps = pp_work.tile([P, 2, F], F32, tag="pswork", name="ps")
        s2ps = pp_work.tile([P, 2], F32, tag="pswork", name="s2ps")
        for half in range(2):
            for k in range(NKT):
                lhsT = ht_sb[k, ch][:, off + half * P:off + (half + 1) * P]
                nc.tensor.matmul(
                    ps[:, half, :], lhsT=lhsT, rhs=wsb[b][:, k, :],
                    start=(k == 0), stop=(k == NKT - 1))
                nc.tensor.matmul(
                    s2ps[:, half:half + 1], lhsT=lhsT,
                    rhs=wa16[b][:, 2 * k + 1:2 * k + 2],
                    start=(k == 0), stop=(k == NKT - 1))
        wh = whp.tile([P, 2, F], F16, tag="wh", name="wh")
        nc.scalar.copy(out=wh, in_=ps)
        s2 = whp.tile([P, 2], F32, tag="s2", name="s2", bufs=6)
        nc.vector.tensor_copy(out=s2, in_=s2ps)

        # masked logits + leaky in ONE custom DVE op per half; exp on Act.
        m = mask_t[b, tp]
        lrp = workp.tile([P, 2, R], F16, tag="lrp", name="lrp", bufs=3)
        for half in range(2):
            nc.vector._custom_dve(
                LOGIT_OP, out=lrp[:, half, :], in0=s1b[b],
                in1=m[:, half * R:(half + 1) * R],
                s0=s2[:, half:half + 1], imm2=ALPHA)
        ptp = pexp.tile([P, 2 * R], F16, tag="ptp", name="ptp", bufs=5)
        nc.scalar.activation(
            out=ptp, in_=lrp.rearrange("p two r -> p (two r)"), func=AF.Exp)
        whs = [wh[:, 0, :], wh[:, 1, :]]
        return whs, ptp

    def consume(b, tp, whs, ptp):
        first, last = (tp == 0), (tp == NPR - 1)
        for half in range(2):
            pt = ptp[:, half * R:(half + 1) * R]
            st = first and half == 0
            sp = last and half == 1
            nc.tensor.matmul(acc[b, 0], lhsT=whs[half][:, 0:P], rhs=pt,
                             start=st, stop=sp)
            nc.tensor.matmul(acc[b, 1], lhsT=whs[half][:, P:F], rhs=pt,
                             start=st, stop=sp)

    def epilogue(b):
        rsps = pp_work.tile([P, R], F32, tag="pswork", name="rsps")
        nc.tensor.matmul(rsps, lhsT=ones16[:, 0:P], rhs=racc[b],
                         start=True, stop=True)
        rb = epip.tile([P, R], F32, tag="rb", name="rb", bufs=1)
        nc.vector.reciprocal(out=rb, in_=rsps)
        for fh in range(2):
            o = epip.tile([P, R], F32, tag="o", name="o")
            nc.vector.scalar_tensor_tensor(
                out=o, in0=acc[b, fh], scalar=1.0, in1=rb,
                op0=ALU.mult, op1=ALU.mult)
            em = epip.tile([P, R], F32, tag="em", name="em", bufs=2)
            nc.scalar.activation(out=em, in_=o, func=AF.Exp)
            t = epip.tile([P, R], F32, tag=f"t_{b}{fh}", name="t", bufs=1)
            # t = min(exp(o), 1) + relu(o)  ==  elu(o) + 1
            nc.vector._custom_dve(ELU1_OP, out=t, in0=em, in1=o)
            tb[b][fh] = t

    items = [(b, tp) for b in BR for tp in range(NPR)]
    inflight = []
    pend = {b: [] for b in BR}       # (tp, ptp) awaiting delayed racc emission
    for b, tp in items:
        inflight.append((b, tp, *produce(b, tp)))
        pend[b].append((tp, inflight[-1][3]))
        if len(pend[b]) > 2:
            emit_racc(b, *pend[b].pop(0))
        if len(inflight) > DELAY:
            bc, tpc, whs, ptp = inflight.pop(0)
            consume(bc, tpc, whs, ptp)
            if tpc == NPR - 1:
                for item in pend[bc]:
                    emit_racc(bc, *item)
                pend[bc].clear()
                epilogue(bc)
    for bc, tpc, whs, ptp in inflight:
        consume(bc, tpc, whs, ptp)
        if tpc == NPR - 1:
            for item in pend[bc]:
                emit_racc(bc, *item)
            pend[bc].clear()
            epilogue(bc)

    for fh in range(2):
        c = epip.tile([P, R], F32, tag="comb", name="comb")
        # c = (t_n - 2) + t_d  ==  elu(o_n) + elu(o_d)
        nc.vector.scalar_tensor_tensor(
            out=c, in0=tb["n"][fh], scalar=-2.0, in1=tb["d"][fh],
            op0=ALU.add, op1=ALU.add)
        nc.sync.dma_start(out=OUT[fh * P:(fh + 1) * P, :], in_=c)


_CACHED = None


def _get_program():
    global _CACHED
    if _CACHED is None:
        _CACHED = build_program()
    return _CACHED


def _prep_inputs(h, adj_n, adj_d, W_n, a1_n, a2_n, W_d, a1_d, a2_d):
    h = np.asarray(h, np.float32)
    hT = np.ascontiguousarray(h.T)
    com = {
        "ht": hT.astype(np.float16),
        "w_n": np.asarray(W_n, np.float32).astype(np.float16),
        "w_d": np.asarray(W_d, np.float32).astype(np.float16),
        "wt_n": np.ascontiguousarray(
            np.asarray(W_n, np.float32).T).astype(np.float16),
        "wt_d": np.ascontiguousarray(
            np.asarray(W_d, np.float32).T).astype(np.float16),
        "a_n": np.concatenate(
            [np.asarray(a1_n, np.float32), np.asarray(a2_n, np.float32)],
            axis=1).astype(np.float16),
        "a_d": np.concatenate(
            [np.asarray(a1_d, np.float32), np.asarray(a2_d, np.float32)],
            axis=1).astype(np.float16),
    }
    adj = {"n": np.asarray(adj_n), "d": np.asarray(adj_d)}
    maps = []
    for c in range(NCORES):
        m = dict(com)
        m["hto"] = np.ascontiguousarray(
            hT[:, c * R:(c + 1) * R]).astype(np.float16)
        for b in BR:
            blk = adj[b][c * R:(c + 1) * R, :]          # [R, N]
            mt = np.where(blk.T > 0, np.float16(0.0), np.float16(MASKB))
            m[f"mt_{b}"] = np.ascontiguousarray(mt.astype(np.float16))
        maps.append(m)
    return maps


def run_on_hw(inputs, trace=False):
    nc = _get_program()
    maps = _prep_inputs(
        inputs["h"], inputs["adj_n"], inputs["adj_d"],
        inputs["W_n"], inputs["a1_n"], inputs["a2_n"],
        inputs["W_d"], inputs["a1_d"], inputs["a2_d"])
    last_err = None
    for attempt in range(3):
        try:
            res = bass_utils.run_bass_kernel_spmd(
                nc, maps, core_ids=list(range(NCORES)), trace=trace)
            break
        except Exception as e:          # transient NRT/axon failures recover
            last_err = e
            import time as _time
            _time.sleep(5)
    else:
        raise last_err
    out = np.concatenate(
        [res.results[c]["out"].T for c in range(NCORES)], axis=0)
    return np.ascontiguousarray(out), res


def kernel(**inputs):
    out, _ = run_on_hw(inputs, trace=False)
    return out


# revision 13
# speedup vs baseline: 1.3647x; 1.0235x over previous
"""DGANet dual-GAT layer on 8 Trainium2 NeuronCores (Bass/Tile), v2.

Math (per branch b in {n, d}):
    Wh = h @ W_b                                  [4096, 256]
    e  = leaky_relu(s1_i + s2_j, 0.2)             s1 = h@(W@a1), s2 = h@(W@a2)
    att = softmax(where(adj>0, e, -9e15), axis=-1)
    f_b = elu(att @ Wh)
Output: f_n + f_d.

Sharding: 1D row-parallel over the 4096 attention rows (512 rows/core).
Each core computes the full Wh and holds its score block transposed,
P^T[j, i] (j on partitions), so att @ Wh contracts over j on the tensor
engine.  The adjacency mask is a host-prepared additive bias (0 or -16384,
fp16): exp underflows masked entries to exactly 0.

v2 vs v1:
  - fp16 end-to-end on the data path (hT/W/masks shipped fp16 from host):
    no fp32->fp32r conversion copies, half the DMA bytes, and the DVE runs
    its 2-byte 2x/4x perf modes on the logit elementwise ops.
  - row-sum of exp moved off the tensor engine: pt tiles are accumulated on
    DVE (delayed by one pair so the DVE queue never blocks the exp chain)
    and a single ones-matmul per branch reduces across partitions.
  - single flat produce/consume stream across both branches (no inter-branch
    PE bubble), epilogue emitted mid-stream.
  - fewer, larger DMAs (packed wt/a/W tiles), masks interleaved with hT
    chunks so the attention pipeline starts ~4us in.
  - output written as [F, R] straight from SBUF (no PE transposes); host
    transposes the per-core block.
"""

from contextlib import ExitStack

import numpy as np

import concourse.bass as bass
import concourse.bacc as bacc
import concourse.mybir as mybir
import concourse.tile as tile
from concourse import bass_utils
import concourse.dve_ops as _D
import concourse.dve_spec as _dsp
from concourse.dve_spec import Spec as _Spec, Src0 as _S0, Src1 as _S1, \
    C0 as _C0, C2 as _C2, One as _One, maxx as _maxx, minn as _minn, \
    relu as _relu, lower as _lower
from concourse.dve_uop import DveOpSpec as _DveOpSpec


def _register_dve_op(name, spec):
    """Register a custom DVE op (idempotent) and return it."""
    for _o in _D.OPS:
        if _o.name == name:
            return _o
    _D._SUB_OPCODE_FOR_NAME[name] = _D._CUSTOM_DVE_ROW_BASE + len(_D.OPS)
    assert _D._SUB_OPCODE_FOR_NAME[name] < 0x20
    has_src1 = _S1 in _dsp.spec_leaves(spec)
    shas = {}
    for _ver in ("v3", "v4"):
        _s = _DveOpSpec(name=name, opcode=_D.get_dve_sub_opcode(name),
                        uops=_lower(spec, ver=_ver), rd1_en=has_src1)
        shas[_ver] = _s.sha(_ver)
    op = _D.DveOp(name, spec, subdim=False, uops_sha=shas)
    _D.OPS.append(op)
    return op


_U = _S0 + _S1 + _C0
# out = leaky_relu(in0 + in1 + s0, imm2): the whole masked-logit elementwise
# (s1 + mask + s2 then leaky) in one DVE instruction.
LOGIT_OP = _register_dve_op("LOGIT_FUSED_DG", _Spec(
    body=_maxx(_U, _U * _C2),
    reference=lambda in0, in1, s0, s1, imm2:
        __import__("numpy").maximum(in0 + in1 + s0,
                                    (in0 + in1 + s0) * imm2).astype("float32")))
# out = min(in0, 1) + relu(in1)  ==  elu(in1) + 1 when in0 == exp(in1)
ELU1_OP = _register_dve_op("ELU1_FUSED_DG", _Spec(
    body=_minn(_S0, _One) + _relu(_S1),
    reference=lambda in0, in1, s0, s1, imm2:
        (__import__("numpy").minimum(in0, 1.0)
         + __import__("numpy").maximum(in1, 0.0)).astype("float32")))

N, FIN, F = 4096, 512, 256
NCORES = 8
R = N // NCORES            # 512 attention rows per core
P = 128                    # partitions
NJT = N // P               # 32 j-tiles
NKT = FIN // P             # 4 fin contraction tiles
NPR = NJT // 2             # 16 j-tile pairs
CH = 4                     # hT column chunks
CHW = N // CH              # 1024 j-cols per chunk
MASKB = -16384.0           # additive mask: exp underflows to 0
ALPHA = 0.2
DELAY = 3                  # produce->consume pipeline depth (in pairs)

F32 = mybir.dt.float32
F16 = mybir.dt.float16
AF = mybir.ActivationFunctionType
ALU = mybir.AluOpType
BR = ("n", "d")


def build_program(reps=None):
    """reps=None: single-shot program (grading path).  reps=K: body wrapped
    in a K-iteration hardware loop, for wall-clock HW timing by slope."""
    nc = bacc.Bacc("TRN2", target_bir_lowering=False, debug=False,
                   num_devices=NCORES)

    hT = nc.dram_tensor("ht", [FIN, N], F16, kind="ExternalInput").ap()
    hTo = nc.dram_tensor("hto", [FIN, R], F16, kind="ExternalInput").ap()
    W = {b: nc.dram_tensor(f"w_{b}", [FIN, F], F16, kind="ExternalInput").ap()
         for b in BR}
    WT = {b: nc.dram_tensor(f"wt_{b}", [F, FIN], F16, kind="ExternalInput").ap()
          for b in BR}
    A = {b: nc.dram_tensor(f"a_{b}", [F, 2], F16, kind="ExternalInput").ap()
         for b in BR}
    MT = {b: nc.dram_tensor(f"mt_{b}", [N, R], F16, kind="ExternalInput").ap()
          for b in BR}
    OUT = nc.dram_tensor("out", [F, R], F32, kind="ExternalOutput").ap()

    with tile.TileContext(nc) as tc:
        if reps is None:
            with ExitStack() as ctx:
                _body(ctx, nc, tc, hT, hTo, W, WT, A, MT, OUT)
        else:
            with tc.For_i(0, reps, 1,
                          hint_engines=(mybir.EngineType.PE,)):
                with ExitStack() as ctx:
                    _body(ctx, nc, tc, hT, hTo, W, WT, A, MT, OUT)
    nc.compile()
    return nc


def _body(ctx, nc, tc, hT, hTo, W, WT, A, MT, OUT):
    consts = ctx.enter_context(tc.tile_pool(name="consts", bufs=1))
    # PSUM banks (8): pp_work 3 + s2 bank + 4 acc banks; rsps borrows a
    # pp_work slot
    pp_work = ctx.enter_context(tc.tile_pool(name="pp_work", bufs=3,
                                             space="PSUM"))
    pp_acc = ctx.enter_context(tc.tile_pool(name="pp_acc", bufs=1,
                                            space="PSUM"))
    maskp = ctx.enter_context(tc.tile_pool(name="maskp", bufs=1))
    whp = ctx.enter_context(tc.tile_pool(name="whp", bufs=6))
    workp = ctx.enter_context(tc.tile_pool(name="workp", bufs=4))
    pexp = ctx.enter_context(tc.tile_pool(name="pexp", bufs=4))
    epip = ctx.enter_context(tc.tile_pool(name="epip", bufs=2))

    ones16 = consts.tile([P, R], F16, tag="ones16")
    nc.vector.memset(ones16, 1.0)

    # PE warm-up on resident constants: ~3.5us so the HAM clock gate reaches
    # 2.4 GHz while the first hT/weight DMAs are still streaming.
    wps = pp_work.tile([P, R], F32, tag="pswork", name="wps")
    for _ in range(8):
        nc.tensor.matmul(wps, lhsT=ones16[:, 0:P], rhs=ones16,
                         start=True, stop=True)

    # ---- DMA issue order (one serialized HWDGE stream) ---------------------
    # needed first: ht chunk 0, branch-n weights, hto; masks interleave after.
    ht_sb = {}

    def dma_ht(ch):
        for k in range(NKT):
            t = consts.tile([P, CHW], F16, tag=f"ht{k}_{ch}")
            nc.sync.dma_start(
                out=t, in_=hT[k * P:(k + 1) * P, ch * CHW:(ch + 1) * CHW])
            ht_sb[k, ch] = t

    dma_ht(0)

    wsb = {}
    wt_sb = {}
    a_sb = {}
    for b in BR:
        t = consts.tile([P, NKT, F], F16, tag=f"w_{b}")
        nc.sync.dma_start(
            out=t, in_=W[b].rearrange("(kt p) f -> p kt f", p=P))
        wsb[b] = t
        wt = consts.tile([P, 2, FIN], F16, tag=f"wt_{b}")
        nc.sync.dma_start(
            out=wt, in_=WT[b].rearrange("(two p) fin -> p two fin", p=P))
        wt_sb[b] = wt
        at = consts.tile([P, 2, 2], F16, tag=f"a_{b}")
        nc.sync.dma_start(
            out=at, in_=A[b].rearrange("(two p) c -> p two c", p=P))
        a_sb[b] = at
        if b == "n":
            hto_t = consts.tile([P, NKT, R], F16, tag="hto")
            nc.sync.dma_start(
                out=hto_t, in_=hTo.rearrange("(kt p) r -> p kt r", p=P))

    mask_t = {}

    def dma_mask(b, tp):
        m = maskp.tile([P, 2 * R], F16, tag=f"m_{b}{tp}", name=f"m{tp}")
        nc.sync.dma_start(
            out=m.rearrange("p (two r) -> p two r", two=2),
            in_=MT[b][2 * tp * P:(2 * tp + 2) * P, :].rearrange(
                "(two p) r -> p two r", two=2))
        mask_t[b, tp] = m

    for tp in range(2):
        dma_mask("n", tp)
    dma_ht(1)
    for tp in range(2, 6):
        dma_mask("n", tp)
    dma_ht(2)
    for tp in range(6, 12):
        dma_mask("n", tp)
    dma_ht(3)
    for tp in range(12, NPR):
        dma_mask("n", tp)
    for tp in range(NPR):
        dma_mask("d", tp)

    # ---- small weight prep: wa = W@a on PE, s1b ----------------------------
    wa_r = {}
    wa16 = {}
    for b in BR:
        ps = pp_work.tile([P, 2 * NKT], F32, tag="pswork", name="wa")
        for m in range(NKT):
            for fk in range(2):
                nc.tensor.matmul(
                    ps[:, 2 * m:2 * m + 2],
                    lhsT=wt_sb[b][:, fk, m * P:(m + 1) * P],
                    rhs=a_sb[b][:, fk, :],
                    start=(fk == 0), stop=(fk == 1))
        wa = consts.tile([P, 2 * NKT], F32, tag=f"wa_{b}")
        nc.vector.tensor_copy(out=wa, in_=ps)
        w16 = consts.tile([P, 2 * NKT], F16, tag=f"wa16_{b}")
        nc.vector.tensor_copy(out=w16, in_=wa)
        wa16[b] = w16
        # wa1 chunk m replicated across 128 cols: stationary operand whose
        # matmul output is s1 already broadcast over partitions.
        reps = []
        for m in range(NKT):
            rt = consts.tile([P, P], F16, tag=f"war_{b}{m}", name=f"war{m}")
            nc.vector.tensor_copy(
                out=rt, in_=wa[:, 2 * m:2 * m + 1].broadcast_to((P, P)))
            reps.append(rt)
        wa_r[b] = reps

    s1b = {}
    for b in BR:
        ps1 = pp_work.tile([P, R], F32, tag="pswork", name="s1")
        for k in range(NKT):
            nc.tensor.matmul(
                ps1, lhsT=wa_r[b][k], rhs=hto_t[:, k, :],
                start=(k == 0), stop=(k == NKT - 1))
        t = consts.tile([P, R], F16, tag=f"s1b_{b}")
        nc.scalar.copy(out=t, in_=ps1)
        s1b[b] = t

    # ---- main loop: flat produce/consume stream over both branches ---------
    acc = {(b, fh): pp_acc.tile([P, R], F32, tag=f"acc_{b}{fh}",
                                name=f"acc_{b}{fh}")
           for b in BR for fh in range(2)}
    s2bank = pp_acc.tile([P, 4 * NPR], F32, tag="s2bank", name="s2bank")
    s2off = {"n": 0, "d": 2 * NPR}
    racc = {b: workp.tile([P, R], F16, tag=f"racc_{b}", name="racc", bufs=1)
            for b in BR}
    tb = {"n": [None, None], "d": [None, None]}

    def emit_racc(b, tp, ptp):
        # delayed one pair behind produce so these adds queue after the next
        # pair's ump/lrp on DVE.  Pool pre-adds the two halves (TensorTensor
        # is the one elementwise op GPSIMD supports, SBUF-only).
        tmp = workp.tile([P, R], F16, tag="rtmp", name="rtmp", bufs=2)
        nc.gpsimd.tensor_tensor(out=tmp, in0=ptp[:, 0:R], in1=ptp[:, R:2 * R],
                                op=ALU.add)
        if tp == 0:
            nc.vector.tensor_copy(out=racc[b], in_=tmp)
        else:
            nc.vector.tensor_tensor(out=racc[b], in0=racc[b], in1=tmp,
                                    op=ALU.add)

    def produce(b, tp):
        jt0 = 2 * tp
        ch, off = divmod(jt0 * P, CHW)
        # Wh pair in ONE psum bank [P, 2, F]; s2 pair accumulates into the
        # shared per-branch columns of one long-lived psum bank.
        ps = pp_work.tile([P, 2, F], F32, tag="pswork", name="ps")
        c0 = s2off[b] + 2 * tp
        for half in range(2):
            for k in range(NKT):
                lhsT = ht_sb[k, ch][:, off + half * P:off + (half + 1) * P]
                nc.tensor.matmul(
                    ps[:, half, :], lhsT=lhsT, rhs=wsb[b][:, k, :],
                    start=(k == 0), stop=(k == NKT - 1))
                nc.tensor.matmul(
                    s2bank[:, c0 + half:c0 + half + 1], lhsT=lhsT,
                    rhs=wa16[b][:, 2 * k + 1:2 * k + 2],
                    start=(k == 0), stop=(k == NKT - 1))
        wh = whp.tile([P, 2, F], F16, tag="wh", name="wh")
        nc.scalar.copy(out=wh, in_=ps)
        s2 = whp.tile([P, 2], F32, tag="s2", name="s2", bufs=8)
        nc.vector.tensor_copy(out=s2, in_=s2bank[:, c0:c0 + 2])

        # masked logits + leaky in ONE custom DVE op per half; exp on Act.
        m = mask_t[b, tp]
        lrp = workp.tile([P, 2, R], F16, tag="lrp", name="lrp", bufs=3)
        for half in range(2):
            nc.vector._custom_dve(
                LOGIT_OP, out=lrp[:, half, :], in0=s1b[b],
                in1=m[:, half * R:(half + 1) * R],
                s0=s2[:, half:half + 1], imm2=ALPHA)
        ptp = pexp.tile([P, 2 * R], F16, tag="ptp", name="ptp", bufs=5)
        nc.scalar.activation(
            out=ptp, in_=lrp.rearrange("p two r -> p (two r)"), func=AF.Exp)
        whs = [wh[:, 0, :], wh[:, 1, :]]
        return whs, ptp

    def consume(b, tp, whs, ptp):
        first, last = (tp == 0), (tp == NPR - 1)
        for half in range(2):
            pt = ptp[:, half * R:(half + 1) * R]
            st = first and half == 0
            sp = last and half == 1
            nc.tensor.matmul(acc[b, 0], lhsT=whs[half][:, 0:P], rhs=pt,
                             start=st, stop=sp)
            nc.tensor.matmul(acc[b, 1], lhsT=whs[half][:, P:F], rhs=pt,
                             start=st, stop=sp)

    def epilogue(b):
        rsps = pp_work.tile([P, R], F32, tag="pswork", name="rsps")
        nc.tensor.matmul(rsps, lhsT=ones16[:, 0:P], rhs=racc[b],
                         start=True, stop=True)
        rb = epip.tile([P, R], F32, tag="rb", name="rb", bufs=1)
        nc.vector.reciprocal(out=rb, in_=rsps)
        for fh in range(2):
            o = epip.tile([P, R], F32, tag="o", name="o")
            nc.vector.scalar_tensor_tensor(
                out=o, in0=acc[b, fh], scalar=1.0, in1=rb,
                op0=ALU.mult, op1=ALU.mult)
            em = epip.tile([P, R], F32, tag="em", name="em", bufs=2)
            nc.scalar.activation(out=em, in_=o, func=AF.Exp)
            t = epip.tile([P, R], F32, tag=f"t_{b}{fh}", name="t", bufs=1)
            # t = min(exp(o), 1) + relu(o)  ==  elu(o) + 1
            nc.vector._custom_dve(ELU1_OP, out=t, in0=em, in1=o)
            tb[b][fh] = t

    items = [(b, tp) for b in BR for tp in range(NPR)]
    inflight = []
    pend = {b: [] for b in BR}       # (tp, ptp) awaiting delayed racc emission
    for b, tp in items:
        inflight.append((b, tp, *produce(b, tp)))
        pend[b].append((tp, inflight[-1][3]))
        if len(pend[b]) > 2:
            emit_racc(b, *pend[b].pop(0))
        if len(inflight) > DELAY:
            bc, tpc, whs, ptp = inflight.pop(0)
            consume(bc, tpc, whs, ptp)
            if tpc == NPR - 1:
                for item in pend[bc]:
                    emit_racc(bc, *item)
                pend[bc].clear()
                epilogue(bc)
    for bc, tpc, whs, ptp in inflight:
        consume(bc, tpc, whs, ptp)
        if tpc == NPR - 1:
            for item in pend[bc]:
                emit_racc(bc, *item)
            pend[bc].clear()
            epilogue(bc)

    for fh in range(2):
        c = epip.tile([P, R], F32, tag="comb", name="comb")
        # c = (t_n - 2) + t_d  ==  elu(o_n) + elu(o_d)
        nc.vector.scalar_tensor_tensor(
            out=c, in0=tb["n"][fh], scalar=-2.0, in1=tb["d"][fh],
            op0=ALU.add, op1=ALU.add)
        nc.sync.dma_start(out=OUT[fh * P:(fh + 1) * P, :], in_=c)


_CACHED = None


def _get_program():
    global _CACHED
    if _CACHED is None:
        _CACHED = build_program()
    return _CACHED


def _prep_inputs(h, adj_n, adj_d, W_n, a1_n, a2_n, W_d, a1_d, a2_d):
    h = np.asarray(h, np.float32)
    hT = np.ascontiguousarray(h.T)
    com = {
        "ht": hT.astype(np.float16),
        "w_n": np.asarray(W_n, np.float32).astype(np.float16),
        "w_d": np.asarray(W_d, np.float32).astype(np.float16),
        "wt_n": np.ascontiguousarray(
            np.asarray(W_n, np.float32).T).astype(np.float16),
        "wt_d": np.ascontiguousarray(
            np.asarray(W_d, np.float32).T).astype(np.float16),
        "a_n": np.concatenate(
            [np.asarray(a1_n, np.float32), np.asarray(a2_n, np.float32)],
            axis=1).astype(np.float16),
        "a_d": np.concatenate(
            [np.asarray(a1_d, np.float32), np.asarray(a2_d, np.float32)],
            axis=1).astype(np.float16),
    }
    adj = {"n": np.asarray(adj_n), "d": np.asarray(adj_d)}
    maps = []
    for c in range(NCORES):
        m = dict(com)
        m["hto"] = np.ascontiguousarray(
            hT[:, c * R:(c + 1) * R]).astype(np.float16)
        for b in BR:
            blk = adj[b][c * R:(c + 1) * R, :]          # [R, N]
            mt = np.where(blk.T > 0, np.float16(0.0), np.float16(MASKB))
            m[f"mt_{b}"] = np.ascontiguousarray(mt.astype(np.float16))
        maps.append(m)
    return maps


def run_on_hw(inputs, trace=False):
    nc = _get_program()
    maps = _prep_inputs(
        inputs["h"], inputs["adj_n"], inputs["adj_d"],
        inputs["W_n"], inputs["a1_n"], inputs["a2_n"],
        inputs["W_d"], inputs["a1_d"], inputs["a2_d"])
    last_err = None
    for attempt in range(3):
        try:
            res = bass_utils.run_bass_kernel_spmd(
                nc, maps, core_ids=list(range(NCORES)), trace=trace)
            break
        except Exception as e:          # transient NRT/axon failures recover
            last_err = e
            import time as _time
            _time.sleep(5)
    else:
        raise last_err
    out = np.concatenate(
        [res.results[c]["out"].T for c in range(NCORES)], axis=0)
    return np.ascontiguousarray(out), res


def kernel(**inputs):
    out, _ = run_on_hw(inputs, trace=False)
    return out
